# revision 28
# baseline (speedup 1.0000x reference)
"""Causal multi-head attention block (B=2, S=2048, M=1024, H=16, D=64) for 8
Trainium2 NeuronCores.

Sharding: tensor-parallel over heads (2 heads per core). Each core computes
QKV for its heads from the full x (bf16), runs causal attention, then two
AllToAlls (one per batch) re-shard z so every core computes its 512 output
rows (64-row interleaved ownership) against the full W_proj. The first
AllToAll is issued at the batch-0 boundary and hides under batch-1
attention; only the second is on the critical path, and half the output
projection runs underneath it. Scores run as fp8e4m3 DoubleRow matmuls
(2x PE rate) with Q/K repacked on-device into a zero-padded 64-partition
pair layout via a DRAM bounce; everything else is bf16. The causal mask is
applied by accumulating a -30000 triangular matrix into the score PSUM via
a bf16 matmul, keeping the DVE off the score->exp->AV critical chain. The
whole kernel is one fine-grained schedule: QKV row-block work is injected
as filler between every attention key-tile, AV matmuls trail their scores
by two tiles, and each block's softmax normalization is deferred into the
next block so the PE never waits on the ACT engine's exp.

Self-contained: hardcodes all shapes; host-side numpy shards/transposes
inputs and reorders/concatenates outputs.
"""

import numpy as np

import concourse.bass as bass
import concourse.bacc as bacc
import concourse.mybir as mybir
import concourse.tile as tile
from concourse.bass_utils import run_bass_kernel_spmd

B, S, M, H, D = 2, 2048, 1024, 16, 64
NC = 8
R = B * S                  # 4096 rows
HPC = H // NC              # 2 heads per core
MC = HPC * D               # 128 m-columns per core
P = 128
RB = 512                   # phase-1 row block
QB = 512                   # phase-2 query block
NRB = R // RB              # 8
NQB = S // QB              # 4 query blocks per batch
NMT = M // P               # 8 m-tiles
NVT = R // P               # 32 V row tiles
ROWS_PC = R // NC          # 512 output rows per core
NEG = -30000.0

f32 = mybir.dt.float32
bf16 = mybir.dt.bfloat16
f8 = mybir.dt.float8e4
AF = mybir.ActivationFunctionType
ALU = mybir.AluOpType

_BUILD_CACHE = {}


def _score_scale():
    sq = TUNE["ws_q"] if TUNE["qmm"] != "bf" else 1.0
    sk = TUNE["ws_k"] if TUNE["kmm"] != "bf" else 1.0
    return sq * sk

TUNE = {"st2_bufs": 2, "b1_st2": 3, "zt_b1": 2, "ex_bufs": 4, "xp_bufs": 2,
        "acc_bufs": 2, "acc_bufs_a": 6, "dma_tp": False, "no_coll": False,
        "wsplit": False, "xt_dmas": 2, "qk_dve": True, "a2a_one": True, "os_split": True, "pe_bc": False, "fp8_qk": True, "av_depth": 2, "z_f8": False, "fill_split": 12, "zsrc": True,
        # fp8 DoubleRow QKV matmuls for q/k: "bf" (bf16), "fp8" (plain),
        # "wres" (plain + weight-quantization-residual second accumulation).
        # Weights are pre-scaled by ws_* host-side; the product scale
        # ws_q*ws_k is divided out inside the exp activation.
        "qmm": "wres", "kmm": "fp8", "ws_q": 256.0, "ws_k": 64.0,
        "qk_fold": "bounce", "zt_batch": True, "a2a_split": True}


def build_nc(with_bias=False, for_sim=False, phases=3, repeat=1):
    key = ("nc", with_bias, for_sim, phases, repeat,
           tuple(sorted(TUNE.items())))
    if key in _BUILD_CACHE:
        return _BUILD_CACHE[key]
    nc = bacc.Bacc("TRN2", target_bir_lowering=False, debug=False,
                   num_devices=1 if for_sim else NC)

    qmm, kmm = TUNE["qmm"], TUNE["kmm"]
    fp8_mm = qmm != "bf" or kmm != "bf"

    xT = nc.dram_tensor("xT", [M, R], bf16, kind="ExternalInput").ap()
    wq = wk = None
    if qmm == "bf":
        wq = nc.dram_tensor("wq", [M, MC], bf16, kind="ExternalInput").ap()
    if kmm == "bf":
        wk = nc.dram_tensor("wk", [M, MC], bf16, kind="ExternalInput").ap()
    wv = nc.dram_tensor("wv", [M, MC], bf16, kind="ExternalInput").ap()
    x8p_v = None
    if fp8_mm:
        # x in fp8, pair layout for DoubleRow: [p, rb, mt, i, r'] with
        # m-dim 256*mt + 2*p + i, row 512*rb + r'; contiguous per (p, rb).
        x8p = nc.dram_tensor("x8p", [P, NRB * 4 * 2 * RB], f8,
                             kind="ExternalInput").ap()
        x8p_v = x8p.rearrange("p (rb mt i r) -> p rb mt i r",
                              rb=NRB, mt=4, i=2)
    w8_d = {}
    w8_grp = {}
    for nm, mode in (("q", qmm), ("k", kmm)):
        if mode != "bf":
            ngrp = 8 if mode == "wres" else 4
            w8_grp[nm] = ngrp
            w8_d[nm] = nc.dram_tensor(
                f"w{nm}8", [P, ngrp * 2 * MC], f8,
                kind="ExternalInput").ap().rearrange(
                "p (g i c) -> p g i c", g=ngrp, i=2)
    bqkv = nc.dram_tensor("bqkv", [P, 3], f32, kind="ExternalInput").ap()
    wp = nc.dram_tensor("wp", [M, M], bf16, kind="ExternalInput").ap()
    maskT = nc.dram_tensor("maskT", [P, P], bf16, kind="ExternalInput").ap()
    ident_d = nc.dram_tensor("ident_d", [P, P], bf16, kind="ExternalInput").ap()

    out = nc.dram_tensor("out", [ROWS_PC, M], f32, kind="ExternalOutput").ap()

    local_coll = for_sim or TUNE["no_coll"]

    with tile.TileContext(nc) as tc:
        with (
            tc.tile_pool(name="cb", bufs=1) as cb,        # constants / persistents
            tc.tile_pool(name="dram", bufs=1, space="DRAM") as dram,
        ):
            # ---- constants ----
            wq_sb = wk_sb = None
            wv_sb = cb.tile([P, NMT, MC], bf16)
            w8_sb = {}
            for nm in ("q", "k"):
                if nm in w8_d:
                    w8_sb[nm] = cb.tile([P, w8_grp[nm], 2, MC], f8,
                                        name=f"w{nm}8_sb")
                    nc.sync.dma_start(w8_sb[nm][:], w8_d[nm][:])
            if qmm == "bf":
                wq_sb = cb.tile([P, NMT, MC], bf16)
                if TUNE["wsplit"]:
                    for mt in range(NMT):
                        nc.sync.dma_start(wq_sb[:, mt, :],
                                          wq[mt * P:(mt + 1) * P, :])
                else:
                    nc.sync.dma_start(
                        wq_sb[:], wq.rearrange("(mt p) d -> p mt d", p=P))
            if kmm == "bf":
                wk_sb = cb.tile([P, NMT, MC], bf16)
                nc.gpsimd.dma_start(
                    wk_sb[:], wk.rearrange("(mt p) d -> p mt d", p=P))
            nc.gpsimd.dma_start(
                wv_sb[:], wv.rearrange("(mt p) d -> p mt d", p=P))
            bias_sb = cb.tile([P, 3], f32)
            if with_bias:
                nc.sync.dma_start(bias_sb[:], bqkv[:])
            maskT_sb = cb.tile([P, P], bf16)
            ident = cb.tile([P, P], bf16)
            nc.gpsimd.dma_start(maskT_sb[:], maskT[:])
            nc.gpsimd.dma_start(ident[:], ident_d[:])

            # ---- persistent activations ----
            fp8_qk = TUNE["fp8_qk"]
            qk_dt = f8 if fp8_qk else bf16
            QT = cb.tile([P, R], qk_dt)       # [2h*64, rows], q pre-scaled
            KT = cb.tile([P, R], qk_dt)
            if fp8_qk:
                # zero-padded pair layout for DoubleRow: head h dims (2p+i)
                # at [64h+p, i, row] for p<32; rows 64h+32..64h+64 stay zero
                # (32-partition DoubleRow crashes TRN2; 64-partition works)
                Q8 = cb.tile([P, 2, R], f8)
                K8 = cb.tile([P, 2, R], f8)
                z8 = nc.inline_tensor(
                    np.zeros((32, 2 * R), dtype=mybir.dt.np(f8)),
                    name="z8").ap()
                for d8 in (Q8, K8):
                    for p0 in (32, 96):
                        nc.gpsimd.dma_start(
                            d8[p0:p0 + 32, :, :].rearrange(
                                "p two r -> p (two r)"), z8[:])
            VA = cb.tile([P, NVT, 65], bf16)   # [V_A | ones]
            VB = cb.tile([P, NVT, P], bf16)    # [ones | 0*63 | V_B]
            z_dt = f8 if TUNE["z_f8"] else bf16
            ZT = cb.tile([P, R], z_dt)
            zt_sb = cb.tile([P, NMT, ROWS_PC], z_dt)   # phase-3 stationary

            nc.vector.memset(VA[:, :, 64:65], 1.0)
            nc.vector.memset(VB[:, :, 0:1], 1.0)
            nc.vector.memset(VB[:, :, 1:64], 0.0)
            ones_sb = cb.tile([P, 64], bf16)
            nc.vector.memset(ones_sb[:], 1.0)

            # phase-3 weights
            wp_sb = cb.tile([P, NMT, M], bf16)

            if TUNE["a2a_split"]:
                a2a_in = [[dram.tile([M, 192], z_dt, name=f"a2a_inA{h}"),
                           dram.tile([M, 64], z_dt, name=f"a2a_inB{h}")]
                          for h in range(2)]
                a2a_out = [[dram.tile([M, 192], z_dt, name=f"a2a_outA{h}"),
                            dram.tile([M, 64], z_dt, name=f"a2a_outB{h}")]
                           for h in range(2)]
            else:
                a2a_in = [dram.tile([M, 256], z_dt, name=f"a2a_in{h}")
                          for h in range(2)]
                a2a_out = [dram.tile([M, 256], z_dt, name=f"a2a_out{h}")
                           for h in range(2)]
            if fp8_qk:
                qb8 = dram.tile([P, R], f8, name="qb8")
                kb8 = dram.tile([P, R], f8, name="kb8")

            def copy_cast(dst, src, which):
                if with_bias:
                    nc.scalar.activation(dst, src, AF.Identity,
                                         bias=bias_sb[:, which:which + 1])
                elif which == 2 or not TUNE["qk_dve"]:
                    nc.scalar.activation(dst, src, AF.Copy)
                else:
                    nc.vector.tensor_copy(dst, src)

            def ph1_ops(rb, ps1, acc_bufs, xp, vp):
                """Phase-1 work for one row block as a list of closures."""
                r0 = rb * RB
                st = {}
                ops = []

                def op_x8():
                    def f():
                        st["x8"] = xp.tile([P, 4, 2, RB], f8, tag="x8",
                                           name="x8")
                        nc.scalar.dma_start(st["x8"][:], x8p_v[:, rb])
                    return f

                def op_xt(i0, n):
                    def f():
                        if "xt" not in st:
                            st["xt"] = xp.tile([P, NMT, RB], bf16, tag="xt",
                                               name="xt")
                        if TUNE["xt_dmas"] <= 2:
                            # one batched descriptor per 4-mt group
                            nc.sync.dma_start(
                                st["xt"][:, i0:i0 + n, :],
                                xT.rearrange("(mt p) r -> p mt r",
                                             p=P)[:, i0:i0 + n, r0:r0 + RB])
                            return
                        for i in range(i0, i0 + n):
                            eng = (nc.scalar if (rb == 0 and i % 2 == 1)
                                   else nc.sync)
                            eng.dma_start(
                                st["xt"][:, i, :],
                                xT[i * P:(i + 1) * P, r0:r0 + RB])
                    return f

                def op_mm8(which, w8sb, g0, g1, ngrp):
                    def f():
                        if ("acc", which) not in st:
                            st[("acc", which)] = ps1.tile(
                                [P, RB], f32, tag="u", name="acc",
                                bufs=acc_bufs)
                        acc = st[("acc", which)]
                        for g in range(g0, g1):
                            nc.tensor.matmul(
                                acc[:], w8sb[:, g, :, :],
                                st["x8"][:, g % 4, :, :],
                                start=(g == 0), stop=(g == ngrp - 1),
                                perf_mode=mybir.MatmulPerfMode.DoubleRow)
                    return f

                def op_mm(which, w_sb, m0, m1):
                    def f():
                        if ("acc", which) not in st:
                            st[("acc", which)] = ps1.tile(
                                [P, RB], f32, tag="u", name="acc",
                                bufs=acc_bufs)
                        acc = st[("acc", which)]
                        for mt in range(m0, m1):
                            nc.tensor.matmul(acc[:], w_sb[:, mt, :],
                                             st["xt"][:, mt, :],
                                             start=(mt == 0),
                                             stop=(mt == NMT - 1))
                    return f

                def op_qk_tail(which, dst):
                    def f():
                        acc = st[("acc", which)]
                        copy_cast(dst[:, r0:r0 + RB], acc[:], which)
                        if not fp8_qk:
                            return
                        d8 = Q8 if which == 0 else K8
                        fold = TUNE["qk_fold"]
                        if fold == "sbuf":
                            # direct SBUF->SBUF partition fold, one DMA
                            nc.sync.dma_start(
                                d8.rearrange("(h p) two r -> h p two r",
                                             h=2)[:, 0:32, :, r0:r0 + RB],
                                dst[:, r0:r0 + RB].rearrange(
                                    "(h p two) r -> h p two r",
                                    h=2, two=2))
                            return
                        db = qb8 if which == 0 else kb8
                        weng = nc.sync
                        reng = nc.gpsimd if rb < 2 else nc.sync
                        weng.dma_start(db[:, r0:r0 + RB],
                                       dst[:, r0:r0 + RB])
                        if fold == "bounce1":
                            reng.dma_start(
                                d8.rearrange("(h p) two r -> h p two r",
                                             h=2)[:, 0:32, :, r0:r0 + RB],
                                db[:, r0:r0 + RB].rearrange(
                                    "(h p two) r -> h p two r",
                                    h=2, two=2))
                            return
                        for h in range(2):
                            reng.dma_start(
                                d8[64 * h:64 * h + 32, :, r0:r0 + RB],
                                db[64 * h:64 * h + 64,
                                   r0:r0 + RB].rearrange(
                                    "(p two) r -> p two r", two=2))
                    return f

                def op_v_tail():
                    def f():
                        vt_sb = vp.tile([P, RB], bf16, tag="vt", name="vt_sb")
                        st["vt"] = vt_sb
                        copy_cast(vt_sb[:], st[("acc", 2)][:], 2)
                    return f

                def op_tp(k0, n):
                    def f():
                        vt_sb = st["vt"]
                        for k in range(k0, k0 + n):
                            t = rb * (RB // P) + k
                            tp = ps1.tile([P, P], bf16, name="tp", tag="u",
                                          bufs=acc_bufs)
                            nc.tensor.transpose(
                                tp[:], vt_sb[:, k * P:(k + 1) * P], ident[:])
                            nc.vector.tensor_copy(VA[:, t, 0:64], tp[:, 0:64])
                            nc.vector.tensor_copy(VB[:, t, 64:128],
                                                  tp[:, 64:128])
                    return f

                dops = ([op_x8()] if fp8_mm else []) + \
                    [op_xt(0, 4), op_xt(4, 4)]
                for which, (mode, w_sb, dst) in enumerate(
                        ((qmm, wq_sb, QT), (kmm, wk_sb, KT),
                         ("bf", wv_sb, None))):
                    if mode == "bf":
                        ops.append(op_mm(which, w_sb, 0, 4))
                        ops.append(op_mm(which, w_sb, 4, 8))
                    else:
                        nm = "qk"[which]
                        ngrp = w8_grp[nm]
                        half = max(ngrp // 2, 1)
                        ops.append(op_mm8(which, w8_sb[nm], 0, half, ngrp))
                        if half < ngrp:
                            ops.append(
                                op_mm8(which, w8_sb[nm], half, ngrp, ngrp))
                    if dst is not None:
                        ops.append(op_qk_tail(which, dst))
                    else:
                        ops.append(op_v_tail())
                        ops.append(op_tp(0, 2))
                        ops.append(op_tp(2, 2))
                return dops, ops

            def emit_ph1(rb, ps1, acc_bufs, xp, vp):
                dops, ops = ph1_ops(rb, ps1, acc_bufs, xp, vp)
                for f in dops + ops:
                    f()

            def emit_ph2(b, qb, ps2, exp_pool, norm_pool, st2_bufs,
                         zt_bufs=2, prev_norm=None, filler=None, last=False):
                gr0 = b * S + qb * QB
                zt_a = ps2.tile([65, QB], f32, tag="zt", bufs=zt_bufs,
                                name="zt_a")
                zt_b = ps2.tile([P, QB], f32, tag="zt", bufs=zt_bufs,
                                name="zt_b")
                nkj = 4 * qb + 4

                def emit_av(t, ex, w, col_off):
                    vt_idx = 16 * b + t
                    for h, (zt_x, vx) in enumerate(((zt_a, VA), (zt_b, VB))):
                        nc.tensor.matmul(
                            zt_x[:, col_off:col_off + w], vx[:, vt_idx, :],
                            ex[:, h, :w],
                            start=(t == 0), stop=(t == nkj - 1),
                            skip_group_check=True)

                pends = []
                depth = TUNE["av_depth"]
                for t in range(nkj):
                    kj0 = 128 * t
                    di = t - 4 * qb
                    if di < 0:
                        col_off, w = 0, QB
                    elif di == 3:
                        col_off, w = 384, 128
                    else:
                        col_off, w = 128 * di, QB - 128 * di
                    st2 = ps2.tile([P, 2 * QB], f32, tag="st2",
                                   bufs=st2_bufs, name="st2")
                    for h in range(2):
                        if fp8_qk:
                            nc.tensor.matmul(
                                st2[:, h * QB:h * QB + w],
                                K8[64 * h:64 * h + 64, :,
                                   b * S + kj0: b * S + kj0 + 128],
                                Q8[64 * h:64 * h + 64, :,
                                   gr0 + col_off: gr0 + col_off + w],
                                start=True, stop=(di < 0),
                                skip_group_check=True,
                                perf_mode=mybir.MatmulPerfMode.DoubleRow)
                        else:
                            hp = slice(64 * h, 64 * h + 64)
                            nc.tensor.matmul(
                                st2[:, h * QB:h * QB + w],
                                KT[hp, b * S + kj0: b * S + kj0 + 128],
                                QT[hp, gr0 + col_off: gr0 + col_off + w],
                                start=True, stop=(di < 0),
                                skip_group_check=True)
                    if di >= 0:
                        for h in range(2):
                            nc.tensor.matmul(
                                st2[:, h * QB:h * QB + 128],
                                maskT_sb[:], ident[:],
                                start=False, stop=True, skip_group_check=True)
                    ex = exp_pool.tile([P, 2, QB], bf16, tag="ex", name="ex")
                    st2v = st2.rearrange("p (h q) -> p h q", h=2)
                    nc.scalar.activation(ex[:, :, :w], st2v[:, :, :w], AF.Exp,
                                         scale=1.0 / _score_scale())
                    if filler is not None:
                        filler(t)
                    pends.append((t, ex, w, col_off))
                    if len(pends) > depth:
                        emit_av(*pends.pop(0))
                    if t == min(1, nkj - 1) and prev_norm is not None:
                        prev_norm()
                for pd in pends:
                    emit_av(*pd)

                recip2 = norm_pool.tile([P, QB], f32, tag="recip",
                                        name="recip")
                nc.vector.reciprocal(recip2[64:65, :], zt_a[64:65, :])
                nc.vector.reciprocal(recip2[0:1, :], zt_b[0:1, :])
                if TUNE["zsrc"]:
                    zsrc = norm_pool.tile([P, QB], f32, tag="zc", name="zc")
                    nc.vector.tensor_copy(zsrc[0:64, :], zt_a[0:64, :])
                    nc.vector.tensor_copy(zsrc[64:128, :], zt_b[64:128, :])
                    za, zb = zsrc[0:64, :], zsrc[64:128, :]
                else:
                    za, zb = zt_a[0:64, :], zt_b[64:128, :]

                def do_norm():
                    rowa = norm_pool.tile([1, QB], f32, tag="rowa",
                                          name="rowa")
                    nc.gpsimd.dma_start(rowa[:], recip2[64:65, :])
                    bca = norm_pool.tile([64, QB], f32, tag="bca",
                                         name="bca")
                    bcb = norm_pool.tile([P, QB], f32, tag="bcb",
                                         name="bcb")
                    nc.gpsimd.partition_broadcast(bca[:], rowa[:],
                                                  channels=64)
                    nc.gpsimd.partition_broadcast(bcb[:], recip2[0:1, :],
                                                  channels=128)
                    with nc.allow_low_precision(reason="fp8 z"):
                        nc.vector.tensor_tensor(
                            ZT[0:64, gr0:gr0 + QB], za, bca[:], ALU.mult)
                        nc.vector.tensor_tensor(
                            ZT[64:128, gr0:gr0 + QB], zb,
                            bcb[64:128, :], ALU.mult)
                    if phases >= 3:
                        if TUNE["a2a_split"]:
                            dst = (a2a_in[b][0][:, qb * 64:(qb + 1) * 64]
                                   if qb < 3 else a2a_in[b][1][:, :])
                        else:
                            dst = a2a_in[b][:, qb * 64:(qb + 1) * 64]
                        nc.sync.dma_start(
                            dst.rearrange("(c p) w -> p c w", c=NC),
                            ZT[:, gr0:gr0 + QB].rearrange(
                                "p (c w) -> p c w", c=NC))
                return do_norm

            def emit_coll(h, part=None):
                if TUNE["a2a_split"]:
                    parts = {"A": (0, 0, 192), "B": (1, 192, 256)}
                    todo = ([parts[part]] if part else
                            [parts["A"], parts["B"]])
                else:
                    todo = [(None, 0, 256)]
                for idx, w0, w1 in todo:
                    src = a2a_in[h][idx] if idx is not None else a2a_in[h]
                    dst = a2a_out[h][idx] if idx is not None else a2a_out[h]
                    if local_coll:
                        nc.sync.dma_start(dst[:], src[:])
                    else:
                        nc.gpsimd.collective_compute(
                            "AllToAll", ALU.bypass,
                            replica_groups=[list(range(NC))],
                            ins=[src.opt()], outs=[dst.opt()],
                        )
                    if TUNE["zt_batch"]:
                        nc.sync.dma_start(
                            zt_sb[:, :, h * 256 + w0:h * 256 + w1],
                            dst.rearrange("(mt p) w -> p mt w", p=P))
                    else:
                        for mt in range(NMT):
                            nc.sync.dma_start(
                                zt_sb[:, mt, h * 256 + w0:h * 256 + w1],
                                dst[mt * P:(mt + 1) * P, :])

            def emit_ph3(rt, out_pool, ps3, tag="o", bufs=4):
                os_ = out_pool.tile([P, M], f32, tag="os", name="os_")
                for nh in range(2):
                    acc = ps3.tile([P, 512], f32, tag=tag, name="acc3",
                                   bufs=bufs)
                    for mt in range(NMT):
                        nc.tensor.matmul(
                            acc[:], zt_sb[:, mt, rt * P:(rt + 1) * P],
                            wp_sb[:, mt, nh * 512:(nh + 1) * 512],
                            start=(mt == 0), stop=(mt == NMT - 1))
                    nc.scalar.activation(os_[:, nh * 512:(nh + 1) * 512],
                                         acc[:], AF.Copy)
                    if TUNE["os_split"]:
                        nc.sync.dma_start(
                            out[rt * P:(rt + 1) * P,
                                nh * 512:(nh + 1) * 512],
                            os_[:, nh * 512:(nh + 1) * 512])
                if not TUNE["os_split"]:
                    nc.sync.dma_start(out[rt * P:(rt + 1) * P, :], os_[:])

            def emit_iter(rep):
                sfx = f"_{rep}"
                with (
                    tc.tile_pool(name="xp" + sfx,
                                 bufs=TUNE["xp_bufs"]) as xp,
                    tc.tile_pool(name="vp" + sfx, bufs=2) as vp,
                    tc.tile_pool(name="ex" + sfx,
                                 bufs=TUNE["ex_bufs"]) as exp_pool,
                    tc.tile_pool(name="np" + sfx, bufs=2) as norm_pool,
                ):
                    if phases < 2:
                        with tc.tile_pool(name="ps1a" + sfx, bufs=1,
                                          space="PSUM") as ps1a:
                            for rb in range(NRB):
                                emit_ph1(rb, ps1a, TUNE["acc_bufs_a"],
                                         xp, vp)
                        return
                    # rb0-1 pure (own PSUM pool, deep acc rotation)
                    with tc.tile_pool(name="ps1a" + sfx, bufs=1,
                                      space="PSUM") as ps1a:
                        for rb in (0, 1):
                            emit_ph1(rb, ps1a, TUNE["acc_bufs_a"], xp, vp)
                    # unified schedule: every ph2 block carries ph1/wp filler
                    # ops; rb k feeds blocks from k-2 on, so emission of rb k
                    # inside block k-2 keeps every dependency satisfied.
                    with tc.tile_pool(name="ps2" + sfx, bufs=1,
                                      space="PSUM") as ps2:
                        def wp_op(mt):
                            def f():
                                nc.gpsimd.dma_start(
                                    wp_sb[:, mt, :],
                                    wp[mt * P:(mt + 1) * P, :])
                            return f

                        rbd = {}
                        rbc = {}
                        for rb in range(2, 8):
                            rbd[rb], rbc[rb] = ph1_ops(
                                rb, ps2, TUNE["acc_bufs"], xp, vp)
                        wops = [wp_op(mt) for mt in range(NMT)] \
                            if rep == 0 else []
                        r6 = rbd[6] + rbc[6]
                        r7 = rbd[7] + rbc[7]
                        fill_by_block = [
                            rbd[2] + rbc[2], rbd[3] + rbc[3],
                            rbd[4] + rbc[4], rbd[5] + rbc[5] + wops,
                            r6[:6], r6[6:] + r7[:4], r7[4:], [],
                        ]
                        blocks = [(0, 0), (0, 1), (0, 2), (0, 3),
                                  (1, 0), (1, 1), (1, 2), (1, 3)]
                        out_pool_cm = tc.tile_pool(name="op" + sfx, bufs=2)
                        out_pool = out_pool_cm.__enter__()
                        pn = None
                        for bi, (b, qb) in enumerate(blocks):
                            q = list(fill_by_block[bi])
                            nt = 4 * qb + 4

                            def filler(t, q=q, nt=nt):
                                k = -(-len(q) // (nt - t)) if t < nt else 0
                                for _ in range(min(k, len(q))):
                                    q.pop(0)()

                            pn = emit_ph2(b, qb, ps2, exp_pool, norm_pool,
                                          TUNE["st2_bufs"],
                                          zt_bufs=2, prev_norm=pn,
                                          filler=filler,
                                          last=(bi == len(blocks) - 1))
                            for f in q:
                                f()
                            if bi == 3 and phases >= 2 and pn is not None:
                                pn()
                                pn = None
                            if bi == 3 and phases >= 3:
                                emit_coll(0)
                            if (bi == 7 and phases >= 3
                                    and TUNE["a2a_split"]):
                                # qb0-2 of batch 1 are normed; exchange them
                                # under the tail of block (1,3)
                                emit_coll(1, "A")
                        if pn is not None:
                            pn()
                        if phases >= 3:
                            if TUNE["a2a_split"]:
                                emit_coll(1, "B")
                            else:
                                emit_coll(1)
                            for rt in (0, 1):
                                emit_ph3(rt, out_pool, ps2, tag="st2",
                                         bufs=TUNE["st2_bufs"])
                    if phases >= 3:
                        with tc.tile_pool(name="ps3" + sfx, bufs=1,
                                          space="PSUM") as ps3:
                            for rt in (2, 3):
                                emit_ph3(rt, out_pool, ps3)
                    out_pool_cm.__exit__(None, None, None)

            for rep in range(repeat):
                emit_iter(rep)

    nc.compile()
    _BUILD_CACHE[key] = nc
    return nc


def _pair_pack_w(Wc, ws, wres, nf8):
    """[M, MC] weight slice -> fp8 pair layout [P, ngrp*2*MC] with m-dim
    256*mt + 2*p + i at [p, mt, i, c]; wres appends the quantization
    residual as groups 4..7 (same ws scale, accumulated in-PSUM)."""
    Ws = np.asarray(Wc, np.float64) * ws
    main = Ws.astype(np.float32).astype(nf8)
    grps = [main]
    if wres:
        grps.append((Ws - main.astype(np.float64))
                    .astype(np.float32).astype(nf8))
    out = [g.reshape(4, P, 2, MC).transpose(1, 0, 2, 3) for g in grps]
    arr = np.concatenate(out, axis=1)
    return np.ascontiguousarray(arr.reshape(P, -1))


def prep_inputs(x, W_attn, b_attn, W_proj, b_proj):
    x = np.asarray(x, dtype=np.float32)
    W_attn = np.asarray(W_attn, dtype=np.float32)
    b_attn = np.asarray(b_attn, dtype=np.float32)
    W_proj = np.asarray(W_proj, dtype=np.float32)
    nbf = mybir.dt.np(bf16)
    nf8 = mybir.dt.np(f8)
    qmm, kmm = TUNE["qmm"], TUNE["kmm"]
    fp8_mm = qmm != "bf" or kmm != "bf"
    SS = _score_scale()

    xT = np.ascontiguousarray(x.reshape(R, M).T).astype(nbf)
    jj = np.arange(P)[None, :]
    pp = np.arange(P)[:, None]
    madd = np.where(jj >= pp, 0.0, NEG * SS).astype(np.float32)  # [key p, q j]
    maskT = np.ascontiguousarray(madd.T).astype(nbf)
    ident = np.eye(P, dtype=np.float32).astype(nbf)
    scale = 1.0 / np.sqrt(D)

    if fp8_mm:
        x8T = xT.astype(np.float32).astype(nf8)   # [M, R] fp8 (from bf16 x)
        # -> [p, rb, mt, i, r']: m = 256*mt + 2*p + i, row = 512*rb + r'
        x8p = np.ascontiguousarray(
            x8T.reshape(4, P, 2, NRB, RB).transpose(1, 3, 0, 2, 4)
            .reshape(P, -1))

    in_maps = []
    for c in range(NC):
        cs = slice(MC * c, MC * (c + 1))
        sq = TUNE["ws_q"] if qmm != "bf" else 1.0
        sk = TUNE["ws_k"] if kmm != "bf" else 1.0
        bq = b_attn[0 * M:1 * M][cs] * scale * sq
        bk = b_attn[1 * M:2 * M][cs] * sk
        bv = b_attn[2 * M:3 * M][cs]
        im = {
            "xT": xT,
            "bqkv": np.ascontiguousarray(np.stack([bq, bk, bv], axis=1)),
            "wp": W_proj.astype(nbf),
            "maskT": maskT, "ident_d": ident,
            "wv": np.ascontiguousarray(
                W_attn[:, 2 * M:3 * M][:, cs]).astype(nbf),
        }
        Wqc = W_attn[:, 0 * M:1 * M][:, cs] * scale
        Wkc = W_attn[:, 1 * M:2 * M][:, cs]
        if qmm == "bf":
            im["wq"] = np.ascontiguousarray(Wqc).astype(nbf)
        else:
            im["wq8"] = _pair_pack_w(Wqc, TUNE["ws_q"], qmm == "wres", nf8)
        if kmm == "bf":
            im["wk"] = np.ascontiguousarray(Wkc).astype(nbf)
        else:
            im["wk8"] = _pair_pack_w(Wkc, TUNE["ws_k"], kmm == "wres", nf8)
        if fp8_mm:
            im["x8p"] = x8p
        in_maps.append(im)
    return in_maps


# local row r on core c -> global row: half = r//256, j = half*4 + (r%256)//64
# (query block index), global = j*512 + c*64 + r%64
def _row_perm():
    perm = np.empty(NC * ROWS_PC, dtype=np.int64)
    for c in range(NC):
        r = np.arange(ROWS_PC)
        j = (r // 256) * 4 + (r % 256) // 64
        g = j * 512 + c * 64 + (r % 64)
        perm[c * ROWS_PC + r] = g
    return perm


_PERM = _row_perm()


def postprocess(results, b_proj):
    out = np.concatenate([results[c]["out"] for c in range(NC)], axis=0)
    full = np.empty_like(out)
    full[_PERM] = out
    full = full + np.asarray(b_proj, dtype=np.float32)[None, :]
    return full.reshape(B, S, M)


def kernel(x, W_attn, b_attn, W_proj, b_proj):
    nc = build_nc(with_bias=bool(np.any(np.asarray(b_attn))))
    in_maps = prep_inputs(x, W_attn, b_attn, W_proj, b_proj)
    res = run_bass_kernel_spmd(nc, in_maps, core_ids=list(range(NC)))
    return postprocess(res.results, b_proj)



# revision 42
# speedup vs baseline: 2.2435x; 2.2435x over previous
"""Causal multi-head attention block (B=2, S=2048, M=1024, H=16, D=64) for 8
Trainium2 NeuronCores.

Sharding: tensor-parallel over heads (2 heads per core). Each core computes
QKV for its heads from the full x (bf16), runs causal attention, then two
AllToAlls (one per batch) re-shard z so every core computes its 512 output
rows (64-row interleaved ownership) against the full W_proj. The first
AllToAll is issued at the batch-0 boundary and hides under batch-1
attention; only the second is on the critical path, and half the output
projection runs underneath it. Scores run as fp8e4m3 DoubleRow matmuls
(2x PE rate) with Q/K repacked on-device into a zero-padded 64-partition
pair layout via a DRAM bounce; everything else is bf16. The causal mask is
applied by accumulating a -30000 triangular matrix into the score PSUM via
a bf16 matmul, keeping the DVE off the score->exp->AV critical chain. The
whole kernel is one fine-grained schedule: QKV row-block work is injected
as filler between every attention key-tile, AV matmuls trail their scores
by two tiles, and each block's softmax normalization is deferred into the
next block so the PE never waits on the ACT engine's exp.

Self-contained: hardcodes all shapes; host-side numpy shards/transposes
inputs and reorders/concatenates outputs.
"""

import numpy as np

import concourse.bass as bass
import concourse.bacc as bacc
import concourse.mybir as mybir
import concourse.tile as tile
from concourse.bass_utils import run_bass_kernel_spmd

B, S, M, H, D = 2, 2048, 1024, 16, 64
NC = 8
R = B * S                  # 4096 rows
HPC = H // NC              # 2 heads per core
MC = HPC * D               # 128 m-columns per core
P = 128
RB = 512                   # phase-1 row block
QB = 512                   # phase-2 query block
NRB = R // RB              # 8
NQB = S // QB              # 4 query blocks per batch
NMT = M // P               # 8 m-tiles
NVT = R // P               # 32 V row tiles
ROWS_PC = R // NC          # 512 output rows per core
NEG = -30000.0

f32 = mybir.dt.float32
bf16 = mybir.dt.bfloat16
f8 = mybir.dt.float8e4
AF = mybir.ActivationFunctionType
ALU = mybir.AluOpType

_BUILD_CACHE = {}


def _score_scale():
    sq = TUNE["ws_q"] if TUNE["qmm"] != "bf" else 1.0
    sk = TUNE["ws_k"] if TUNE["kmm"] != "bf" else 1.0
    return sq * sk

TUNE = {"st2_bufs": 2, "b1_st2": 3, "zt_b1": 2, "ex_bufs": 4, "xp_bufs": 2,
        "acc_bufs": 2, "acc_bufs_a": 6, "dma_tp": False, "no_coll": False,
        "wsplit": False, "xt_dmas": 2, "qk_dve": True, "a2a_one": True, "os_split": True, "pe_bc": False, "fp8_qk": True, "av_depth": 2, "z_f8": False, "fill_split": 12, "zsrc": True,
        # fp8 DoubleRow QKV matmuls for q/k: "bf" (bf16), "fp8" (plain),
        # "wres" (plain + weight-quantization-residual second accumulation).
        # Weights are pre-scaled by ws_* host-side; the product scale
        # ws_q*ws_k is divided out inside the exp activation.
        "qmm": "wres", "kmm": "fp8", "ws_q": 256.0, "ws_k": 64.0,
        "qk_fold": "bounce", "zt_batch": True, "a2a_split": True,
        "ld_eng": "sync", "out_bf": True}


def build_nc(with_bias=False, for_sim=False, phases=3, repeat=1):
    key = ("nc", with_bias, for_sim, phases, repeat,
           tuple(sorted(TUNE.items())))
    if key in _BUILD_CACHE:
        return _BUILD_CACHE[key]
    nc = bacc.Bacc("TRN2", target_bir_lowering=False, debug=False,
                   num_devices=1 if for_sim else NC)

    qmm, kmm = TUNE["qmm"], TUNE["kmm"]
    fp8_mm = qmm != "bf" or kmm != "bf"

    xT = nc.dram_tensor("xT", [M, R], bf16, kind="ExternalInput").ap()
    wq = wk = None
    if qmm == "bf":
        wq = nc.dram_tensor("wq", [M, MC], bf16, kind="ExternalInput").ap()
    if kmm == "bf":
        wk = nc.dram_tensor("wk", [M, MC], bf16, kind="ExternalInput").ap()
    wv = nc.dram_tensor("wv", [M, MC], bf16, kind="ExternalInput").ap()
    x8p_v = None
    if fp8_mm:
        # x in fp8, pair layout for DoubleRow: [p, rb, mt, i, r'] with
        # m-dim 256*mt + 2*p + i, row 512*rb + r'; contiguous per (p, rb).
        x8p = nc.dram_tensor("x8p", [P, NRB * 4 * 2 * RB], f8,
                             kind="ExternalInput").ap()
        x8p_v = x8p.rearrange("p (rb mt i r) -> p rb mt i r",
                              rb=NRB, mt=4, i=2)
    w8_d = {}
    w8_grp = {}
    for nm, mode in (("q", qmm), ("k", kmm)):
        if mode != "bf":
            ngrp = 8 if mode == "wres" else 4
            w8_grp[nm] = ngrp
            w8_d[nm] = nc.dram_tensor(
                f"w{nm}8", [P, ngrp * 2 * MC], f8,
                kind="ExternalInput").ap().rearrange(
                "p (g i c) -> p g i c", g=ngrp, i=2)
    bqkv = nc.dram_tensor("bqkv", [P, 3], f32, kind="ExternalInput").ap()
    wp = nc.dram_tensor("wp", [M, M], bf16, kind="ExternalInput").ap()
    maskT = nc.dram_tensor("maskT", [P, P], bf16, kind="ExternalInput").ap()
    ident_d = nc.dram_tensor("ident_d", [P, P], bf16, kind="ExternalInput").ap()

    out_dt = bf16 if TUNE["out_bf"] else f32
    out = nc.dram_tensor("out", [ROWS_PC, M], out_dt,
                         kind="ExternalOutput").ap()

    local_coll = for_sim or TUNE["no_coll"]

    with tile.TileContext(nc) as tc:
        with (
            tc.tile_pool(name="cb", bufs=1) as cb,        # constants / persistents
            tc.tile_pool(name="dram", bufs=1, space="DRAM") as dram,
        ):
            # ---- constants ----
            wq_sb = wk_sb = None
            wv_sb = cb.tile([P, NMT, MC], bf16)
            w8_sb = {}
            for nm in ("q", "k"):
                if nm in w8_d:
                    w8_sb[nm] = cb.tile([P, w8_grp[nm], 2, MC], f8,
                                        name=f"w{nm}8_sb")
                    nc.sync.dma_start(w8_sb[nm][:], w8_d[nm][:])
            if qmm == "bf":
                wq_sb = cb.tile([P, NMT, MC], bf16)
                if TUNE["wsplit"]:
                    for mt in range(NMT):
                        nc.sync.dma_start(wq_sb[:, mt, :],
                                          wq[mt * P:(mt + 1) * P, :])
                else:
                    nc.sync.dma_start(
                        wq_sb[:], wq.rearrange("(mt p) d -> p mt d", p=P))
            if kmm == "bf":
                wk_sb = cb.tile([P, NMT, MC], bf16)
                nc.gpsimd.dma_start(
                    wk_sb[:], wk.rearrange("(mt p) d -> p mt d", p=P))
            nc.gpsimd.dma_start(
                wv_sb[:], wv.rearrange("(mt p) d -> p mt d", p=P))
            bias_sb = cb.tile([P, 3], f32)
            if with_bias:
                nc.sync.dma_start(bias_sb[:], bqkv[:])
            maskT_sb = cb.tile([P, P], bf16)
            ident = cb.tile([P, P], bf16)
            nc.gpsimd.dma_start(maskT_sb[:], maskT[:])
            nc.gpsimd.dma_start(ident[:], ident_d[:])

            # ---- persistent activations ----
            fp8_qk = TUNE["fp8_qk"]
            qk_dt = f8 if fp8_qk else bf16
            QT = cb.tile([P, R], qk_dt)       # [2h*64, rows], q pre-scaled
            KT = cb.tile([P, R], qk_dt)
            if fp8_qk:
                # zero-padded pair layout for DoubleRow: head h dims (2p+i)
                # at [64h+p, i, row] for p<32; rows 64h+32..64h+64 stay zero
                # (32-partition DoubleRow crashes TRN2; 64-partition works)
                Q8 = cb.tile([P, 2, R], f8)
                K8 = cb.tile([P, 2, R], f8)
                z8 = nc.inline_tensor(
                    np.zeros((32, 2 * R), dtype=mybir.dt.np(f8)),
                    name="z8").ap()
                for d8 in (Q8, K8):
                    for p0 in (32, 96):
                        nc.gpsimd.dma_start(
                            d8[p0:p0 + 32, :, :].rearrange(
                                "p two r -> p (two r)"), z8[:])
            VA = cb.tile([P, NVT, 65], bf16)   # [V_A | ones]
            VB = cb.tile([P, NVT, P], bf16)    # [ones | 0*63 | V_B]
            z_dt = f8 if TUNE["z_f8"] else bf16
            ZT = cb.tile([P, R], z_dt)
            zt_sb = cb.tile([P, NMT, ROWS_PC], z_dt)   # phase-3 stationary

            nc.vector.memset(VA[:, :, 64:65], 1.0)
            nc.vector.memset(VB[:, :, 0:1], 1.0)
            nc.vector.memset(VB[:, :, 1:64], 0.0)
            ones_sb = cb.tile([P, 64], bf16)
            nc.vector.memset(ones_sb[:], 1.0)

            # phase-3 weights
            wp_sb = cb.tile([P, NMT, M], bf16)

            if TUNE["a2a_split"]:
                a2a_in = [[dram.tile([M, 192], z_dt, name=f"a2a_inA{h}"),
                           dram.tile([M, 64], z_dt, name=f"a2a_inB{h}")]
                          for h in range(2)]
                a2a_out = [[dram.tile([M, 192], z_dt, name=f"a2a_outA{h}"),
                            dram.tile([M, 64], z_dt, name=f"a2a_outB{h}")]
                           for h in range(2)]
            else:
                a2a_in = [dram.tile([M, 256], z_dt, name=f"a2a_in{h}")
                          for h in range(2)]
                a2a_out = [dram.tile([M, 256], z_dt, name=f"a2a_out{h}")
                           for h in range(2)]
            if fp8_qk:
                qb8 = dram.tile([P, R], f8, name="qb8")
                kb8 = dram.tile([P, R], f8, name="kb8")

            def copy_cast(dst, src, which):
                if with_bias:
                    nc.scalar.activation(dst, src, AF.Identity,
                                         bias=bias_sb[:, which:which + 1])
                elif which == 2 or not TUNE["qk_dve"]:
                    nc.scalar.activation(dst, src, AF.Copy)
                else:
                    nc.vector.tensor_copy(dst, src)

            def ph1_ops(rb, ps1, acc_bufs, xp, vp):
                """Phase-1 work for one row block as a list of closures."""
                r0 = rb * RB
                st = {}
                ops = []

                def op_x8():
                    def f():
                        st["x8"] = xp.tile([P, 4, 2, RB], f8, tag="x8",
                                           name="x8")
                        eng = (nc.scalar if rb < 2 and
                               TUNE["ld_eng"] == "split" else nc.sync)
                        eng.dma_start(st["x8"][:], x8p_v[:, rb])
                    return f

                def op_xt(i0, n):
                    def f():
                        if "xt" not in st:
                            st["xt"] = xp.tile([P, NMT, RB], bf16, tag="xt",
                                               name="xt")
                        if TUNE["xt_dmas"] <= 2:
                            # one batched descriptor per 4-mt group. rb<2
                            # loads ride the scalar queue (exp-free window at
                            # the iteration boundary) so the rb0/rb1 fold
                            # chain on sync isn't stuck behind them.
                            eng = (nc.scalar if rb < 2 and
                                   TUNE["ld_eng"] == "split" else nc.sync)
                            eng.dma_start(
                                st["xt"][:, i0:i0 + n, :],
                                xT.rearrange("(mt p) r -> p mt r",
                                             p=P)[:, i0:i0 + n, r0:r0 + RB])
                            return
                        for i in range(i0, i0 + n):
                            eng = (nc.scalar if (rb == 0 and i % 2 == 1)
                                   else nc.sync)
                            eng.dma_start(
                                st["xt"][:, i, :],
                                xT[i * P:(i + 1) * P, r0:r0 + RB])
                    return f

                def op_mm8(which, w8sb, g0, g1, ngrp):
                    def f():
                        if ("acc", which) not in st:
                            st[("acc", which)] = ps1.tile(
                                [P, RB], f32, tag="u", name="acc",
                                bufs=acc_bufs)
                        acc = st[("acc", which)]
                        for g in range(g0, g1):
                            nc.tensor.matmul(
                                acc[:], w8sb[:, g, :, :],
                                st["x8"][:, g % 4, :, :],
                                start=(g == 0), stop=(g == ngrp - 1),
                                perf_mode=mybir.MatmulPerfMode.DoubleRow)
                    return f

                def op_mm(which, w_sb, m0, m1):
                    def f():
                        if ("acc", which) not in st:
                            st[("acc", which)] = ps1.tile(
                                [P, RB], f32, tag="u", name="acc",
                                bufs=acc_bufs)
                        acc = st[("acc", which)]
                        for mt in range(m0, m1):
                            nc.tensor.matmul(acc[:], w_sb[:, mt, :],
                                             st["xt"][:, mt, :],
                                             start=(mt == 0),
                                             stop=(mt == NMT - 1))
                    return f

                def op_qk_tail(which, dst):
                    def f():
                        acc = st[("acc", which)]
                        copy_cast(dst[:, r0:r0 + RB], acc[:], which)
                        if not fp8_qk:
                            return
                        d8 = Q8 if which == 0 else K8
                        fold = TUNE["qk_fold"]
                        reng = nc.gpsimd if rb < 2 else nc.sync
                        if fold == "sbuf":
                            # direct SBUF->SBUF partition fold (same proven
                            # per-head AP as the bounce read, DRAM hop gone)
                            for h in range(2):
                                reng.dma_start(
                                    d8[64 * h:64 * h + 32, :, r0:r0 + RB],
                                    dst[64 * h:64 * h + 64,
                                        r0:r0 + RB].rearrange(
                                        "(p two) r -> p two r", two=2))
                            return
                        db = qb8 if which == 0 else kb8
                        nc.sync.dma_start(db[:, r0:r0 + RB],
                                          dst[:, r0:r0 + RB])
                        for h in range(2):
                            reng.dma_start(
                                d8[64 * h:64 * h + 32, :, r0:r0 + RB],
                                db[64 * h:64 * h + 64,
                                   r0:r0 + RB].rearrange(
                                    "(p two) r -> p two r", two=2))
                    return f

                def op_v_tail():
                    def f():
                        vt_sb = vp.tile([P, RB], bf16, tag="vt", name="vt_sb")
                        st["vt"] = vt_sb
                        copy_cast(vt_sb[:], st[("acc", 2)][:], 2)
                    return f

                def op_tp(k0, n):
                    def f():
                        vt_sb = st["vt"]
                        for k in range(k0, k0 + n):
                            t = rb * (RB // P) + k
                            tp = ps1.tile([P, P], bf16, name="tp", tag="u",
                                          bufs=acc_bufs)
                            nc.tensor.transpose(
                                tp[:], vt_sb[:, k * P:(k + 1) * P], ident[:])
                            nc.vector.tensor_copy(VA[:, t, 0:64], tp[:, 0:64])
                            nc.vector.tensor_copy(VB[:, t, 64:128],
                                                  tp[:, 64:128])
                    return f

                dops = ([op_x8()] if fp8_mm else []) + \
                    [op_xt(0, 4), op_xt(4, 4)]
                for which, (mode, w_sb, dst) in enumerate(
                        ((qmm, wq_sb, QT), (kmm, wk_sb, KT),
                         ("bf", wv_sb, None))):
                    if mode == "bf":
                        ops.append(op_mm(which, w_sb, 0, 4))
                        ops.append(op_mm(which, w_sb, 4, 8))
                    else:
                        nm = "qk"[which]
                        ngrp = w8_grp[nm]
                        half = max(ngrp // 2, 1)
                        ops.append(op_mm8(which, w8_sb[nm], 0, half, ngrp))
                        if half < ngrp:
                            ops.append(
                                op_mm8(which, w8_sb[nm], half, ngrp, ngrp))
                    if dst is not None:
                        ops.append(op_qk_tail(which, dst))
                    else:
                        ops.append(op_v_tail())
                        ops.append(op_tp(0, 2))
                        ops.append(op_tp(2, 2))
                return dops, ops

            def emit_ph1(rb, ps1, acc_bufs, xp, vp):
                dops, ops = ph1_ops(rb, ps1, acc_bufs, xp, vp)
                for f in dops + ops:
                    f()

            def emit_ph2(b, qb, ps2, exp_pool, norm_pool, st2_bufs,
                         zt_bufs=2, prev_norm=None, filler=None, last=False):
                gr0 = b * S + qb * QB
                zt_a = ps2.tile([65, QB], f32, tag="zt", bufs=zt_bufs,
                                name="zt_a")
                zt_b = ps2.tile([P, QB], f32, tag="zt", bufs=zt_bufs,
                                name="zt_b")
                nkj = 4 * qb + 4

                def emit_av(t, ex, w, col_off):
                    vt_idx = 16 * b + t
                    for h, (zt_x, vx) in enumerate(((zt_a, VA), (zt_b, VB))):
                        nc.tensor.matmul(
                            zt_x[:, col_off:col_off + w], vx[:, vt_idx, :],
                            ex[:, h, :w],
                            start=(t == 0), stop=(t == nkj - 1),
                            skip_group_check=True)

                pends = []
                depth = TUNE["av_depth"]
                for t in range(nkj):
                    kj0 = 128 * t
                    di = t - 4 * qb
                    if di < 0:
                        col_off, w = 0, QB
                    elif di == 3:
                        col_off, w = 384, 128
                    else:
                        col_off, w = 128 * di, QB - 128 * di
                    st2 = ps2.tile([P, 2 * QB], f32, tag="st2",
                                   bufs=st2_bufs, name="st2")
                    for h in range(2):
                        if fp8_qk:
                            nc.tensor.matmul(
                                st2[:, h * QB:h * QB + w],
                                K8[64 * h:64 * h + 64, :,
                                   b * S + kj0: b * S + kj0 + 128],
                                Q8[64 * h:64 * h + 64, :,
                                   gr0 + col_off: gr0 + col_off + w],
                                start=True, stop=(di < 0),
                                skip_group_check=True,
                                perf_mode=mybir.MatmulPerfMode.DoubleRow)
                        else:
                            hp = slice(64 * h, 64 * h + 64)
                            nc.tensor.matmul(
                                st2[:, h * QB:h * QB + w],
                                KT[hp, b * S + kj0: b * S + kj0 + 128],
                                QT[hp, gr0 + col_off: gr0 + col_off + w],
                                start=True, stop=(di < 0),
                                skip_group_check=True)
                    if di >= 0:
                        for h in range(2):
                            nc.tensor.matmul(
                                st2[:, h * QB:h * QB + 128],
                                maskT_sb[:], ident[:],
                                start=False, stop=True, skip_group_check=True)
                    ex = exp_pool.tile([P, 2, QB], bf16, tag="ex", name="ex")
                    st2v = st2.rearrange("p (h q) -> p h q", h=2)
                    nc.scalar.activation(ex[:, :, :w], st2v[:, :, :w], AF.Exp,
                                         scale=1.0 / _score_scale())
                    if filler is not None:
                        filler(t)
                    pends.append((t, ex, w, col_off))
                    if len(pends) > depth:
                        emit_av(*pends.pop(0))
                    if t == min(1, nkj - 1) and prev_norm is not None:
                        prev_norm()
                for pd in pends:
                    emit_av(*pd)

                recip2 = norm_pool.tile([P, QB], f32, tag="recip",
                                        name="recip")
                nc.vector.reciprocal(recip2[64:65, :], zt_a[64:65, :])
                nc.vector.reciprocal(recip2[0:1, :], zt_b[0:1, :])
                if TUNE["zsrc"]:
                    zsrc = norm_pool.tile([P, QB], f32, tag="zc", name="zc")
                    nc.vector.tensor_copy(zsrc[0:64, :], zt_a[0:64, :])
                    nc.vector.tensor_copy(zsrc[64:128, :], zt_b[64:128, :])
                    za, zb = zsrc[0:64, :], zsrc[64:128, :]
                else:
                    za, zb = zt_a[0:64, :], zt_b[64:128, :]

                def do_norm():
                    rowa = norm_pool.tile([1, QB], f32, tag="rowa",
                                          name="rowa")
                    nc.gpsimd.dma_start(rowa[:], recip2[64:65, :])
                    bca = norm_pool.tile([64, QB], f32, tag="bca",
                                         name="bca")
                    bcb = norm_pool.tile([P, QB], f32, tag="bcb",
                                         name="bcb")
                    nc.gpsimd.partition_broadcast(bca[:], rowa[:],
                                                  channels=64)
                    nc.gpsimd.partition_broadcast(bcb[:], recip2[0:1, :],
                                                  channels=128)
                    with nc.allow_low_precision(reason="fp8 z"):
                        nc.vector.tensor_tensor(
                            ZT[0:64, gr0:gr0 + QB], za, bca[:], ALU.mult)
                        nc.vector.tensor_tensor(
                            ZT[64:128, gr0:gr0 + QB], zb,
                            bcb[64:128, :], ALU.mult)
                    if phases >= 3:
                        if TUNE["a2a_split"]:
                            dst = (a2a_in[b][0][:, qb * 64:(qb + 1) * 64]
                                   if qb < 3 else a2a_in[b][1][:, :])
                        else:
                            dst = a2a_in[b][:, qb * 64:(qb + 1) * 64]
                        nc.sync.dma_start(
                            dst.rearrange("(c p) w -> p c w", c=NC),
                            ZT[:, gr0:gr0 + QB].rearrange(
                                "p (c w) -> p c w", c=NC))
                return do_norm

            def emit_coll(h, part=None):
                if TUNE["a2a_split"]:
                    parts = {"A": (0, 0, 192), "B": (1, 192, 256)}
                    todo = ([parts[part]] if part else
                            [parts["A"], parts["B"]])
                else:
                    todo = [(None, 0, 256)]
                for idx, w0, w1 in todo:
                    src = a2a_in[h][idx] if idx is not None else a2a_in[h]
                    dst = a2a_out[h][idx] if idx is not None else a2a_out[h]
                    if local_coll:
                        nc.sync.dma_start(dst[:], src[:])
                    else:
                        nc.gpsimd.collective_compute(
                            "AllToAll", ALU.bypass,
                            replica_groups=[list(range(NC))],
                            ins=[src.opt()], outs=[dst.opt()],
                        )
                    if TUNE["zt_batch"]:
                        nc.sync.dma_start(
                            zt_sb[:, :, h * 256 + w0:h * 256 + w1],
                            dst.rearrange("(mt p) w -> p mt w", p=P))
                    else:
                        for mt in range(NMT):
                            nc.sync.dma_start(
                                zt_sb[:, mt, h * 256 + w0:h * 256 + w1],
                                dst[mt * P:(mt + 1) * P, :])

            def emit_ph3(rt, out_pool, ps3, tag="o", bufs=4):
                os_ = out_pool.tile([P, M], out_dt, tag="os", name="os_")
                for nh in range(2):
                    acc = ps3.tile([P, 512], f32, tag=tag, name="acc3",
                                   bufs=bufs)
                    for mt in range(NMT):
                        nc.tensor.matmul(
                            acc[:], zt_sb[:, mt, rt * P:(rt + 1) * P],
                            wp_sb[:, mt, nh * 512:(nh + 1) * 512],
                            start=(mt == 0), stop=(mt == NMT - 1))
                    nc.scalar.activation(os_[:, nh * 512:(nh + 1) * 512],
                                         acc[:], AF.Copy)
                    if TUNE["os_split"]:
                        nc.sync.dma_start(
                            out[rt * P:(rt + 1) * P,
                                nh * 512:(nh + 1) * 512],
                            os_[:, nh * 512:(nh + 1) * 512])
                if not TUNE["os_split"]:
                    nc.sync.dma_start(out[rt * P:(rt + 1) * P, :], os_[:])

            def emit_iter(rep):
                sfx = f"_{rep}"
                with (
                    tc.tile_pool(name="xp" + sfx,
                                 bufs=TUNE["xp_bufs"]) as xp,
                    tc.tile_pool(name="vp" + sfx, bufs=2) as vp,
                    tc.tile_pool(name="ex" + sfx,
                                 bufs=TUNE["ex_bufs"]) as exp_pool,
                    tc.tile_pool(name="np" + sfx, bufs=2) as norm_pool,
                ):
                    if phases < 2:
                        with tc.tile_pool(name="ps1a" + sfx, bufs=1,
                                          space="PSUM") as ps1a:
                            for rb in range(NRB):
                                emit_ph1(rb, ps1a, TUNE["acc_bufs_a"],
                                         xp, vp)
                        return
                    # rb0-1 pure (own PSUM pool, deep acc rotation)
                    with tc.tile_pool(name="ps1a" + sfx, bufs=1,
                                      space="PSUM") as ps1a:
                        for rb in (0, 1):
                            emit_ph1(rb, ps1a, TUNE["acc_bufs_a"], xp, vp)
                    # unified schedule: every ph2 block carries ph1/wp filler
                    # ops; rb k feeds blocks from k-2 on, so emission of rb k
                    # inside block k-2 keeps every dependency satisfied.
                    with tc.tile_pool(name="ps2" + sfx, bufs=1,
                                      space="PSUM") as ps2:
                        def wp_op(mt):
                            def f():
                                nc.gpsimd.dma_start(
                                    wp_sb[:, mt, :],
                                    wp[mt * P:(mt + 1) * P, :])
                            return f

                        rbd = {}
                        rbc = {}
                        for rb in range(2, 8):
                            rbd[rb], rbc[rb] = ph1_ops(
                                rb, ps2, TUNE["acc_bufs"], xp, vp)
                        wops = [wp_op(mt) for mt in range(NMT)] \
                            if rep == 0 else []
                        r6 = rbd[6] + rbc[6]
                        r7 = rbd[7] + rbc[7]
                        fill_by_block = [
                            rbd[2] + rbc[2], rbd[3] + rbc[3],
                            rbd[4] + rbc[4], rbd[5] + rbc[5] + wops,
                            r6[:6], r6[6:] + r7[:4], r7[4:], [],
                        ]
                        blocks = [(0, 0), (0, 1), (0, 2), (0, 3),
                                  (1, 0), (1, 1), (1, 2), (1, 3)]
                        out_pool_cm = tc.tile_pool(name="op" + sfx, bufs=2)
                        out_pool = out_pool_cm.__enter__()
                        pn = None
                        for bi, (b, qb) in enumerate(blocks):
                            q = list(fill_by_block[bi])
                            nt = 4 * qb + 4

                            def filler(t, q=q, nt=nt):
                                k = -(-len(q) // (nt - t)) if t < nt else 0
                                for _ in range(min(k, len(q))):
                                    q.pop(0)()

                            pn = emit_ph2(b, qb, ps2, exp_pool, norm_pool,
                                          TUNE["st2_bufs"],
                                          zt_bufs=2, prev_norm=pn,
                                          filler=filler,
                                          last=(bi == len(blocks) - 1))
                            for f in q:
                                f()
                            if bi == 3 and phases >= 2 and pn is not None:
                                pn()
                                pn = None
                            if bi == 3 and phases >= 3:
                                emit_coll(0)
                            if (bi == 7 and phases >= 3
                                    and TUNE["a2a_split"]):
                                # qb0-2 of batch 1 are normed; exchange them
                                # under the tail of block (1,3)
                                emit_coll(1, "A")
                        if pn is not None:
                            pn()
                        if phases >= 3:
                            if TUNE["a2a_split"]:
                                emit_coll(1, "B")
                            else:
                                emit_coll(1)
                            for rt in (0, 1):
                                emit_ph3(rt, out_pool, ps2, tag="st2",
                                         bufs=TUNE["st2_bufs"])
                    if phases >= 3:
                        with tc.tile_pool(name="ps3" + sfx, bufs=1,
                                          space="PSUM") as ps3:
                            for rt in (2, 3):
                                emit_ph3(rt, out_pool, ps3)
                    out_pool_cm.__exit__(None, None, None)

            for rep in range(repeat):
                emit_iter(rep)

    nc.compile()
    _BUILD_CACHE[key] = nc
    return nc


def _pair_pack_w(Wc, ws, wres, nf8):
    """[M, MC] weight slice -> fp8 pair layout [P, ngrp*2*MC] with m-dim
    256*mt + 2*p + i at [p, mt, i, c]; wres appends the quantization
    residual as groups 4..7 (same ws scale, accumulated in-PSUM)."""
    Ws = np.asarray(Wc, np.float64) * ws
    main = Ws.astype(np.float32).astype(nf8)
    grps = [main]
    if wres:
        grps.append((Ws - main.astype(np.float64))
                    .astype(np.float32).astype(nf8))
    out = [g.reshape(4, P, 2, MC).transpose(1, 0, 2, 3) for g in grps]
    arr = np.concatenate(out, axis=1)
    return np.ascontiguousarray(arr.reshape(P, -1))


def prep_inputs(x, W_attn, b_attn, W_proj, b_proj):
    x = np.asarray(x, dtype=np.float32)
    W_attn = np.asarray(W_attn, dtype=np.float32)
    b_attn = np.asarray(b_attn, dtype=np.float32)
    W_proj = np.asarray(W_proj, dtype=np.float32)
    nbf = mybir.dt.np(bf16)
    nf8 = mybir.dt.np(f8)
    qmm, kmm = TUNE["qmm"], TUNE["kmm"]
    fp8_mm = qmm != "bf" or kmm != "bf"
    SS = _score_scale()

    xT = np.ascontiguousarray(x.reshape(R, M).T).astype(nbf)
    jj = np.arange(P)[None, :]
    pp = np.arange(P)[:, None]
    madd = np.where(jj >= pp, 0.0, NEG * SS).astype(np.float32)  # [key p, q j]
    maskT = np.ascontiguousarray(madd.T).astype(nbf)
    ident = np.eye(P, dtype=np.float32).astype(nbf)
    scale = 1.0 / np.sqrt(D)

    if fp8_mm:
        x8T = xT.astype(np.float32).astype(nf8)   # [M, R] fp8 (from bf16 x)
        # -> [p, rb, mt, i, r']: m = 256*mt + 2*p + i, row = 512*rb + r'
        x8p = np.ascontiguousarray(
            x8T.reshape(4, P, 2, NRB, RB).transpose(1, 3, 0, 2, 4)
            .reshape(P, -1))

    in_maps = []
    for c in range(NC):
        cs = slice(MC * c, MC * (c + 1))
        sq = TUNE["ws_q"] if qmm != "bf" else 1.0
        sk = TUNE["ws_k"] if kmm != "bf" else 1.0
        bq = b_attn[0 * M:1 * M][cs] * scale * sq
        bk = b_attn[1 * M:2 * M][cs] * sk
        bv = b_attn[2 * M:3 * M][cs]
        im = {
            "xT": xT,
            "bqkv": np.ascontiguousarray(np.stack([bq, bk, bv], axis=1)),
            "wp": W_proj.astype(nbf),
            "maskT": maskT, "ident_d": ident,
            "wv": np.ascontiguousarray(
                W_attn[:, 2 * M:3 * M][:, cs]).astype(nbf),
        }
        Wqc = W_attn[:, 0 * M:1 * M][:, cs] * scale
        Wkc = W_attn[:, 1 * M:2 * M][:, cs]
        if qmm == "bf":
            im["wq"] = np.ascontiguousarray(Wqc).astype(nbf)
        else:
            im["wq8"] = _pair_pack_w(Wqc, TUNE["ws_q"], qmm == "wres", nf8)
        if kmm == "bf":
            im["wk"] = np.ascontiguousarray(Wkc).astype(nbf)
        else:
            im["wk8"] = _pair_pack_w(Wkc, TUNE["ws_k"], kmm == "wres", nf8)
        if fp8_mm:
            im["x8p"] = x8p
        in_maps.append(im)
    return in_maps


# local row r on core c -> global row: half = r//256, j = half*4 + (r%256)//64
# (query block index), global = j*512 + c*64 + r%64
def _row_perm():
    perm = np.empty(NC * ROWS_PC, dtype=np.int64)
    for c in range(NC):
        r = np.arange(ROWS_PC)
        j = (r // 256) * 4 + (r % 256) // 64
        g = j * 512 + c * 64 + (r % 64)
        perm[c * ROWS_PC + r] = g
    return perm


_PERM = _row_perm()


def postprocess(results, b_proj):
    out = np.concatenate(
        [np.asarray(results[c]["out"], dtype=np.float32)
         for c in range(NC)], axis=0)
    full = np.empty_like(out)
    full[_PERM] = out
    full = full + np.asarray(b_proj, dtype=np.float32)[None, :]
    return full.reshape(B, S, M)


def kernel(x, W_attn, b_attn, W_proj, b_proj):
    nc = build_nc(with_bias=bool(np.any(np.asarray(b_attn))))
    in_maps = prep_inputs(x, W_attn, b_attn, W_proj, b_proj)
    res = run_bass_kernel_spmd(nc, in_maps, core_ids=list(range(NC)))
    return postprocess(res.results, b_proj)

